# revision 34
# baseline (speedup 1.0000x reference)
"""NonLocalAttention2D Trainium2 kernel (v17).

Data-parallel over batch N=8: one image per NeuronCore.

Per-core math (xh: (C=128, HW=4096) fp16, cast from fp32 x on the host so
only 1MB of input DMA gates the prologue and no on-chip casts are needed;
the residual uses fp16 x, error ~3e-4 of output scale):
  kv   = [Wv|Wk].T @ xh              (80, 4096)  PE fp16 (v rows 0:64, k 64:80)
  pool = maxpool2x2(kv)              (80, 1024)  DVE: two strided max-reduces
  A    = [Wq.T; Wq@bk].T @ [K; 1]    (128, 1024) PE fp16 (bias folded via ones
                                     row 80 of kvh), DVE copy -> ab fp16
  bqk  = k.T @ bq, ebqk = exp(.)     (128, 8)    PE + ACT (bias bk.bq)
  vaugT= [vT*ebqk | ebqk]            (128, 8*65) PE transpose f16 + DVE -> bf16
  s_cb = ab_c.T @ xh_b               (128k,512q) PE fp16 -> psum
  attn = exp(s): tiles 1-3 ACT Exp; tile 0 DVE Schraudolph
         (int16(s*128/ln2 + 16252.5) bitcast bf16, ~2% rel err that cancels
         in the softmax normalization; GpSimd cannot read PSUM so only
         ACT/DVE can exp). Tile 0 on DVE keeps the attn tile's last writer
         on ACT, so av never waits on the DVE queue.
  av   = vaugT.T @ attn  (accum 8c)  (65, 512)   PE bf16; row 64 = denom
  r    = recip_approx_fast(denom)    (1, 512)    DVE (input staged to SBUF)
  R65  = broadcast r over 65 parts:  PE ones-matmul (rb bf16 via ACT copy,
         psum -> R65s via ACT copy), issued one iteration before use. A
         DRAM-bounce broadcast serializes the sync DMA queue - avoid.
  ao   = av * R65 -> fp16            (65, 512)   DVE
  fin  = [g*Wo; g*bo'].T @ ao        (128, 512)  PE fp16, same stage as ao
  out  = fin + xh_b                  (128, 512)  DVE -> DMA out

Pipeline: 3-deep (scores/exp | av+recip+broadcast | ao+fin+residual+store),
block-0 scores/exps interleaved into the kv/pool prologue chunk by chunk
(sc0 tiles in ps_fin so the proj ring in ps_sc never waits on exps; vt/bqk
in ps_av so iteration 1 never waits on block-0 exps). Input DMAs dispatch
from both hardware DMA queues (sync + scalar). Last two blocks run their
chain in 256-col halves with the staging copies on the by-then-idle ACT.

PSUM budget (8 banks): ps_sc 2x[128,1024] scores ring = 4, ps_av
2x[128,512] av/vt/bqk = 2, ps_fin 2x[128,512] fin/sc0/a_ps/R65p = 2.
"""

import sys

if "/opt/trn_rl_repo" not in sys.path:
    sys.path.insert(0, "/opt/trn_rl_repo")

import numpy as np

import concourse.bacc as bacc
import concourse.bass as bass
import concourse.tile as tile
from concourse import bass_utils, mybir

F32 = mybir.dt.float32
F16 = mybir.dt.float16
BF16 = mybir.dt.bfloat16
I16 = mybir.dt.int16

C = 128          # channels
HW = 4096        # 64*64 pixels
L = 1024         # pooled keys (32*32)
D = 16           # attn dim
DV = 64          # value dim
KV = 80          # kv projection out width (v rows 0:64, k rows 64:80)
QB = 512         # q-block size
NB = HW // QB    # 8 q blocks
KC = 128         # keys per chunk
NCH = L // KC    # 8 key chunks
NCORES = 8
# wb: ident64 | wqt17 (rows 64:81) | bq (rows 64:80) | wfin | wkv
W16 = DV + C + 1 + C + KV
WF32 = 1                   # bkbq

# Schraudolph exp -> bf16 bits: bits = trunc(s * 2^7/ln2 + (127*2^7 - 4 + 0.5))
SCH_A = 128.0 / 0.6931471805599453
SCH_B = 16252.5


def build_kernel():
    nc = bacc.Bacc("TRN2", target_bir_lowering=False, debug=False)

    xh_d = nc.dram_tensor("xh", (C, HW), F16, kind="ExternalInput").ap()
    wb_d = nc.dram_tensor("wb", (C, W16), F16, kind="ExternalInput").ap()
    baux_d = nc.dram_tensor("baux", (C, WF32), F32, kind="ExternalInput").ap()
    out_d = nc.dram_tensor("out", (C, HW), F32, kind="ExternalOutput").ap()

    from contextlib import ExitStack

    with tile.TileContext(nc) as tc, ExitStack() as ctx:
        singles = ctx.enter_context(tc.tile_pool(name="singles", bufs=1))
        s1_pool = ctx.enter_context(tc.tile_pool(name="s1", bufs=4))
        attn_pool = ctx.enter_context(tc.tile_pool(name="attn", bufs=2))
        r_pool = ctx.enter_context(tc.tile_pool(name="r", bufs=2))
        ao_pool = ctx.enter_context(tc.tile_pool(name="ao", bufs=2))
        out_pool = ctx.enter_context(tc.tile_pool(name="outp", bufs=3))

        ps_sc = ctx.enter_context(tc.tile_pool(name="ps_sc", bufs=2, space="PSUM"))
        ps_av = ctx.enter_context(tc.tile_pool(name="ps_av", bufs=2, space="PSUM"))
        ps_fin = ctx.enter_context(tc.tile_pool(name="ps_fin", bufs=2, space="PSUM"))

        # ---- SBUF singles ----
        wb = singles.tile([C, W16], F16, tag="wb")
        xh = singles.tile([C, HW], F16, tag="xh")
        kvh = singles.tile([KV + 1, L], F16, tag="kvh")  # v 0:64, k 64:80, ones 80
        ab = singles.tile([C, L], F16, tag="ab")
        ones65 = singles.tile([1, DV + 1], BF16, tag="ones")
        baux = singles.tile([C, WF32], F32, tag="baux")

        identh = wb[0:DV, 0:DV]
        w_qt17 = wb[DV : KV + 1, DV : DV + C]            # rows 64:81
        b_q64 = wb[DV : DV + D, DV + C : DV + C + 1]     # rows 64:80
        w_fin = wb[0 : DV + 1, DV + C + 1 : DV + 2 * C + 1]
        w_kv = wb[:, DV + 2 * C + 1 : DV + 2 * C + 1 + KV]
        bkbq = baux[:, 0:1]

        # ---- input DMAs: dispatch from both DMA-capable hardware queues
        # (sync, scalar/ACT) in parallel; piece 0 + weights first ----
        nc.sync.dma_start(out=xh[:, 0:QB], in_=xh_d[:, 0:QB])
        nc.scalar.dma_start(out=wb, in_=wb_d)
        nc.sync.dma_start(out=xh[:, QB : 2 * QB], in_=xh_d[:, QB : 2 * QB])
        nc.scalar.dma_start(out=xh[:, 2 * QB : 3 * QB], in_=xh_d[:, 2 * QB : 3 * QB])
        nc.sync.dma_start(out=xh[:, 3 * QB : 4 * QB], in_=xh_d[:, 3 * QB : 4 * QB])
        nc.scalar.dma_start(out=xh[:, 4 * QB : 5 * QB], in_=xh_d[:, 4 * QB : 5 * QB])
        nc.sync.dma_start(out=xh[:, 5 * QB : 6 * QB], in_=xh_d[:, 5 * QB : 6 * QB])
        nc.scalar.dma_start(out=xh[:, 6 * QB : 7 * QB], in_=xh_d[:, 6 * QB : 7 * QB])
        nc.sync.dma_start(out=xh[:, 7 * QB : 8 * QB], in_=xh_d[:, 7 * QB : 8 * QB])
        nc.scalar.dma_start(out=baux, in_=baux_d)

        nc.vector.memset(ones65, 1.0)
        # ones row (partition 80) for the A-matmul bias fold; whole-tile
        # memset (start partition must be 0), pool overwrites rows 0:80
        nc.gpsimd.memset(kvh, 1.0)

        attn0 = attn_pool.tile([KC, NCH * QB], BF16, tag="attn")
        vt_t = ps_av.tile([C, QB], F32, tag="av")  # 8x(128,64) vT chunks
        vt16 = vt_t.bitcast(F16)
        bqk_t = ps_av.tile([C, QB], F32, tag="av")  # cols 0:8 used

        def late_tail(c):
            # A_c matmul (bias folded via kvh ones row), ab copy, block-0
            # scores + 512-wide exp. sc0 tiles live in ps_fin so the proj
            # ring (ps_sc) never waits on block-0 exps.
            csl = slice(c * KC, (c + 1) * KC)
            a_ps = ps_fin.tile([C, QB], F32, tag="fin", name=f"a{c}")
            nc.tensor.matmul(
                a_ps[:, 0:KC], lhsT=w_qt17, rhs=kvh[DV : KV + 1, csl],
                start=True, stop=True, tile_position=(DV, 0),
            )
            nc.vector.tensor_copy(ab[:, csl], a_ps[:, 0:KC])
            sc0c = ps_fin.tile([KC, QB], F32, tag="fin", name=f"sc0_{c}")
            nc.tensor.matmul(
                sc0c[:, :],
                lhsT=ab[:, csl],
                rhs=xh[:, 0:QB],
                start=True,
                stop=True,
            )
            nc.scalar.activation(
                attn0[:, c * QB : (c + 1) * QB],
                sc0c[:, :],
                mybir.ActivationFunctionType.Exp,
            )
            nc.tensor.matmul(
                bqk_t[:, c : c + 1], lhsT=kvh[DV : DV + D, csl], rhs=b_q64,
                start=True, stop=True, tile_position=(DV, 0),
            )
            nc.tensor.transpose(
                vt16[:, c * DV : (c + 1) * DV], kvh[0:DV, csl], identh
            )

        # ---- prologue: kv proj + pool chain, block-0 scores interleaved ----
        proj = None
        for c in range(NCH):
            j = c % 2
            if j == 0:
                proj = ps_sc.tile([KC, 2 * QB], F32, tag="sc", name=f"proj{c}")
            sl = slice(c * QB, (c + 1) * QB)
            nc.tensor.matmul(
                proj[:KV, j * QB : (j + 1) * QB],
                lhsT=w_kv,
                rhs=xh[:, sl],
                start=True,
                stop=True,
            )
            csl = slice(c * KC, (c + 1) * KC)
            # 2x2 maxpool via two DVE reduces (w-pairs from psum, then h-pairs)
            pv = proj[:KV, j * QB : (j + 1) * QB].rearrange(
                "p (w two) -> p w two", two=2
            )
            s1 = s1_pool.tile([KV, 256], F32, tag="s1")
            nc.vector.tensor_reduce(
                s1[:, :], pv, mybir.AxisListType.X, mybir.AluOpType.max
            )
            sv = s1.rearrange("p (h two w) -> p h w two", h=4, two=2)
            nc.vector.tensor_reduce(
                kvh[:KV, csl], sv, mybir.AxisListType.X, mybir.AluOpType.max
            )
            if c >= 1:
                late_tail(c - 1)
        late_tail(NCH - 1)
        # (defer_kv_aux is emitted right below, after its definition)

        ebqk = singles.tile([KC, NCH], F32, tag="ebqk")
        vaug = singles.tile([KC, NCH * (DV + 1)], BF16, tag="vaug")

        # ebqk + vaug assembly (vt/bqk matmuls already ran inside late_tail)
        nc.scalar.activation(
            ebqk[:, :], bqk_t[:, 0:NCH],
            mybir.ActivationFunctionType.Exp, bias=bkbq,
        )
        for c in range(NCH):
            base = c * (DV + 1)
            nc.vector.tensor_scalar_mul(
                vaug[:, base : base + DV],
                vt16[:, c * DV : (c + 1) * DV],
                ebqk[:, c : c + 1],
            )
            nc.vector.tensor_copy(
                vaug[:, base + DV : base + DV + 1], ebqk[:, c : c + 1]
            )

        # ---- main loop: 4-deep software pipeline (block 0 prefilled) ----
        # iter i: PE [sc(i) x8 | av(i-1) x8 | fin(i-3)]
        #         ACT [exp(i) tiles 0-2], DVE [schraudolph exp tile 3,
        #              dn+recip(i-1), ao-mul(i-2), residual-add(i-3)]
        #         DMA [r bounce (i-2), out (i-3)]
        attn_t, av_t, r_t, R65s_t, ao_t = {}, {}, {}, {}, {}
        attn_t[0] = attn0

        for i in range(1, NB + 3):
            b_sc = i          # scores + exp
            b_av = i - 1      # av accumulation + recip + bounce dispatch
            b_r = i - 2       # ao mul + fin + residual + store

            if b_sc < NB:
                qsl = slice(b_sc * QB, (b_sc + 1) * QB)
                attn = attn_pool.tile([KC, NCH * QB], BF16, tag="attn")
                attn_t[b_sc] = attn
                attn16 = attn.bitcast(I16)
                for t in range(4):
                    sc = ps_sc.tile([KC, 2 * QB], F32, tag="sc")
                    for j in range(2):
                        cc = 2 * t + j
                        nc.tensor.matmul(
                            sc[:, j * QB : (j + 1) * QB],
                            lhsT=ab[:, cc * KC : (cc + 1) * KC],
                            rhs=xh[:, qsl],
                            start=True,
                            stop=True,
                        )
                    # interleave av MMs of previous block between score tiles
                    if t == 1 and 0 <= b_av < NB:
                        _av_mms(nc, ps_av, av_t, vaug, attn_t, b_av, 0, 4)
                    if t == 2 and 0 <= b_av < NB:
                        _av_mms(nc, ps_av, av_t, vaug, attn_t, b_av, 4, 8)
                    if t > 0:
                        nc.scalar.activation(
                            attn[:, t * 2 * QB : (t + 1) * 2 * QB],
                            sc[:, :],
                            mybir.ActivationFunctionType.Exp,
                        )
                    else:
                        # Schraudolph exp on DVE: bf16 bits via int16 affine
                        nc.vector.tensor_scalar(
                            attn16[:, t * 2 * QB : (t + 1) * 2 * QB],
                            sc[:, :],
                            SCH_A,
                            SCH_B,
                            mybir.AluOpType.mult,
                            mybir.AluOpType.add,
                        )
                if b_sc == NB - 1:
                    # last block: start av(7) chunks 0-3 as soon as its first
                    # exps land (rest in the next iteration)
                    _av_mms(nc, ps_av, av_t, vaug, attn_t, b_sc, 0, 4)
            elif 0 <= b_av < NB:
                c0 = 4 if b_av == NB - 1 else 0
                _av_mms(nc, ps_av, av_t, vaug, attn_t, b_av, c0, 8)

            if 0 <= b_av < NB:
                # recip of denominators as soon as av(b_av) stops
                # (custom-DVE recip must read SBUF: stage the psum row first)
                dn = r_pool.tile([1, QB], F32, tag="dn", name=f"dn{b_av}")
                r = r_pool.tile([1, QB], F32, tag="r", name=f"r{b_av}")
                nh = 2 if b_av >= NB - 2 else 1
                for h in range(nh):
                    hs = slice(h * QB // nh, (h + 1) * QB // nh)
                    if nh == 2:  # tail: stage on the by-then-idle ACT
                        nc.scalar.copy(dn[:, hs], av_t[b_av][DV : DV + 1, hs])
                    else:
                        nc.vector.tensor_copy(dn[:, hs], av_t[b_av][DV : DV + 1, hs])
                    nc.vector.reciprocal_approx_fast(r[:, hs], dn[:, hs])
                r_t[b_av] = r
                if b_av < NB - 2:
                    # broadcast r over 65 partitions via PE ones-matmul, a
                    # full iteration before ao-mul needs it (no DMA bounce:
                    # the sync queue then only carries output stores)
                    R65s = r_pool.tile(
                        [DV + 1, QB], F32, tag="R65s", name=f"R65s{b_av}"
                    )
                    rb = r_pool.tile([1, QB], BF16, tag="rb", name=f"rb{b_av}")
                    R65p = ps_fin.tile([C, QB], F32, tag="fin")
                    nc.gpsimd.tensor_copy(rb[:, :], r[:, :])
                    nc.tensor.matmul(
                        R65p[0 : DV + 1, :], lhsT=ones65, rhs=rb[:, :],
                        start=True, stop=True,
                    )
                    nc.scalar.copy(R65s[:, :], R65p[0 : DV + 1, :])
                    R65s_t[b_av] = R65s

            if 0 <= b_r < NB:
                # ao + fin + residual + store all in one stage: the bounce
                # was dispatched a full iteration ago so R65s has landed
                if b_r >= NB - 2:
                    # tail blocks: low-latency PE ones-matmul broadcast,
                    # 256-wide halves, copies on the by-then-idle ACT
                    R65s = r_pool.tile(
                        [DV + 1, QB], F32, tag="R65s", name=f"R65s{b_r}"
                    )
                    rb = r_pool.tile([1, QB], BF16, tag="rb", name=f"rb{b_r}")
                    R65p = ps_fin.tile([C, QB], F32, tag="fin")
                    for h in range(2):
                        hs = slice(h * 256, (h + 1) * 256)
                        nc.scalar.copy(rb[:, hs], r_t[b_r][:, hs])
                        nc.tensor.matmul(
                            R65p[0 : DV + 1, hs], lhsT=ones65, rhs=rb[:, hs],
                            start=True, stop=True,
                        )
                        nc.scalar.copy(R65s[:, hs], R65p[0 : DV + 1, hs])
                    R65s_t[b_r] = R65s
                ao = ao_pool.tile([DV + 1, QB], F16, tag="ao")
                fin = ps_fin.tile([C, QB], F32, tag="fin")
                o = out_pool.tile([C, QB], F32, tag="o")
                if b_r >= NB - 2:
                    for h in range(2):
                        hs = slice(h * 256, (h + 1) * 256)
                        qh = slice(b_r * QB + h * 256, b_r * QB + (h + 1) * 256)
                        nc.vector.tensor_mul(
                            ao[:, hs], av_t[b_r][:, hs], R65s_t[b_r][:, hs]
                        )
                        nc.tensor.matmul(
                            fin[:, hs], lhsT=w_fin, rhs=ao[:, hs],
                            start=True, stop=True,
                        )
                        nc.vector.tensor_add(o[:, hs], fin[:, hs], xh[:, qh])
                        nc.sync.dma_start(out=out_d[:, qh], in_=o[:, hs])
                else:
                    qsl = slice(b_r * QB, (b_r + 1) * QB)
                    nc.vector.tensor_mul(
                        ao[:, :], av_t[b_r][:, :], R65s_t[b_r][:, :]
                    )
                    nc.tensor.matmul(
                        fin[:, :], lhsT=w_fin, rhs=ao[:, :],
                        start=True, stop=True,
                    )
                    nc.vector.tensor_add(o[:, :], fin[:, :], xh[:, qsl])
                    nc.sync.dma_start(out=out_d[:, qsl], in_=o[:, :])

    nc.compile()
    return nc


def _av_mms(nc, ps_av, av_t, vaug, attn_t, b, c0, c1):
    if b not in av_t:
        av_t[b] = ps_av.tile([DV + 1, QB], F32, tag="av", name=f"av{b}")
    av = av_t[b]
    attn = attn_t[b]
    for c in range(c0, c1):
        base = c * (DV + 1)
        nc.tensor.matmul(
            av[:, :],
            lhsT=vaug[:, base : base + DV + 1],
            rhs=attn[:, c * QB : (c + 1) * QB],
            start=(c == 0),
            stop=(c == NCH - 1),
        )


def prep_weights(Wq, bq, Wk, bk, Wv, bv, Wo, bo, gamma):
    g = np.float32(np.asarray(gamma))
    Wq, Wk, Wv, Wo = (np.asarray(a, np.float32) for a in (Wq, Wk, Wv, Wo))
    bq_, bk_, bv_, bo_ = (np.asarray(a, np.float32) for a in (bq, bk, bv, bo))
    wb = np.zeros((C, W16), np.float16)
    wb[0:DV, 0:DV] = np.eye(DV, dtype=np.float16)
    wb[DV : DV + D, DV : DV + C] = Wq.T.astype(np.float16)
    wb[KV, DV : DV + C] = (Wq @ bk_).astype(np.float16)  # bias row (ones fold)
    wb[DV : DV + D, DV + C] = bq_.astype(np.float16)
    wb[0:DV, DV + C + 1 : DV + 2 * C + 1] = (g * Wo).astype(np.float16)
    # bo' = bo + Wo.T bv  (v-bias folded host-side)
    wb[DV, DV + C + 1 : DV + 2 * C + 1] = (g * (bo_ + Wo.T @ bv_)).astype(
        np.float16
    )
    wb[:, DV + 2 * C + 1 : DV + 2 * C + 1 + DV] = Wv.astype(np.float16)
    wb[:, DV + 2 * C + 1 + DV : DV + 2 * C + 1 + KV] = Wk.astype(np.float16)
    baux = np.zeros((C, WF32), np.float32)
    baux[:, 0] = np.float32(bk_ @ bq_)  # bqk scalar bias
    return np.ascontiguousarray(wb), np.ascontiguousarray(baux)


_NC_CACHE = {}


def kernel(x, Wq, bq, Wk, bk, Wv, bv, Wo, bo, gamma):
    x = np.asarray(x, dtype=np.float32)
    N = x.shape[0]
    assert x.shape == (N, C, 64, 64) and N == NCORES
    wb, baux = prep_weights(Wq, bq, Wk, bk, Wv, bv, Wo, bo, gamma)

    if "nc" not in _NC_CACHE:
        _NC_CACHE["nc"] = build_kernel()
    nc = _NC_CACHE["nc"]

    in_maps = []
    for i in range(N):
        in_maps.append(
            {
                "xh": np.ascontiguousarray(x[i].reshape(C, HW).astype(np.float16)),
                "wb": wb,
                "baux": baux,
            }
        )
    res = bass_utils.run_bass_kernel_spmd(nc, in_maps, core_ids=list(range(N)))
    out = np.stack([res.results[i]["out"].reshape(C, 64, 64) for i in range(N)])
    return out.astype(np.float32)


if __name__ == "__main__":
    print("built", build_kernel())


# revision 35
# speedup vs baseline: 1.0785x; 1.0785x over previous
"""NonLocalAttention2D Trainium2 kernel (v17).

Data-parallel over batch N=8: one image per NeuronCore.

Per-core math (xh: (C=128, HW=4096) fp16, cast from fp32 x on the host so
only 1MB of input DMA gates the prologue and no on-chip casts are needed;
the residual uses fp16 x, error ~3e-4 of output scale):
  kv   = [Wv|Wk].T @ xh              (80, 4096)  PE fp16 (v rows 0:64, k 64:80)
  pool = maxpool2x2(kv)              (80, 1024)  DVE: two strided max-reduces
  A    = [Wq.T; Wq@bk].T @ [K; 1]    (128, 1024) PE fp16 (bias folded via ones
                                     row 80 of kvh), DVE copy -> ab fp16
  bqk  = k.T @ bq, ebqk = exp(.)     (128, 8)    PE + ACT (bias bk.bq)
  vaugT= [vT*ebqk | ebqk]            (128, 8*65) PE transpose f16 + DVE -> bf16
  s_cb = ab_c.T @ xh_b               (128k,512q) PE fp16 -> psum
  attn = exp(s): tiles 1-3 ACT Exp; tile 0 DVE Schraudolph
         (int16(s*128/ln2 + 16252.5) bitcast bf16, ~2% rel err that cancels
         in the softmax normalization; GpSimd cannot read PSUM so only
         ACT/DVE can exp). Tile 0 on DVE keeps the attn tile's last writer
         on ACT, so av never waits on the DVE queue.
  av   = vaugT.T @ attn  (accum 8c)  (65, 512)   PE bf16; row 64 = denom
  r    = recip_approx_fast(denom)    (1, 512)    DVE (input staged to SBUF)
  R65  = broadcast r over 65 parts:  PE ones-matmul (rb bf16 via ACT copy,
         psum -> R65s via ACT copy), issued one iteration before use. A
         DRAM-bounce broadcast serializes the sync DMA queue - avoid.
  ao   = av * R65 -> fp16            (65, 512)   DVE
  fin  = [g*Wo; g*bo'].T @ ao        (128, 512)  PE fp16, same stage as ao
  out  = fin + xh_b                  (128, 512)  DVE -> DMA out

Pipeline: 3-deep (scores/exp | av+recip+broadcast | ao+fin+residual+store),
block-0 scores/exps interleaved into the kv/pool prologue chunk by chunk
(sc0 tiles in ps_fin so the proj ring in ps_sc never waits on exps; vt/bqk
in ps_av so iteration 1 never waits on block-0 exps). Input DMAs dispatch
from both hardware DMA queues (sync + scalar). Last two blocks run their
chain in 256-col halves with the staging copies on the by-then-idle ACT.

PSUM budget (8 banks): ps_sc 2x[128,1024] scores ring = 4, ps_av
2x[128,512] av/vt/bqk = 2, ps_fin 2x[128,512] fin/sc0/a_ps/R65p = 2.
"""

import sys

if "/opt/trn_rl_repo" not in sys.path:
    sys.path.insert(0, "/opt/trn_rl_repo")

import numpy as np

import concourse.bacc as bacc
import concourse.bass as bass
import concourse.tile as tile
from concourse import bass_utils, mybir

F32 = mybir.dt.float32
F16 = mybir.dt.float16
BF16 = mybir.dt.bfloat16
I16 = mybir.dt.int16

C = 128          # channels
HW = 4096        # 64*64 pixels
L = 1024         # pooled keys (32*32)
D = 16           # attn dim
DV = 64          # value dim
KV = 80          # kv projection out width (v rows 0:64, k rows 64:80)
QB = 512         # q-block size
NB = HW // QB    # 8 q blocks
KC = 128         # keys per chunk
NCH = L // KC    # 8 key chunks
NCORES = 8
# wb: ident64 | wqt17 (rows 64:81) | bq (rows 64:80) | wfin | wkv
W16 = DV + C + 1 + C + KV
WF32 = 1                   # bkbq

# Schraudolph exp -> bf16 bits: bits = trunc(s * 2^7/ln2 + (127*2^7 - 4 + 0.5))
SCH_A = 128.0 / 0.6931471805599453
SCH_B = 16252.5


def build_kernel():
    nc = bacc.Bacc("TRN2", target_bir_lowering=False, debug=False)

    xh_d = nc.dram_tensor("xh", (C, HW), F16, kind="ExternalInput").ap()
    wb_d = nc.dram_tensor("wb", (C, W16), F16, kind="ExternalInput").ap()
    baux_d = nc.dram_tensor("baux", (C, WF32), F32, kind="ExternalInput").ap()
    out_d = nc.dram_tensor("out", (C, HW), F32, kind="ExternalOutput").ap()

    from contextlib import ExitStack

    with tile.TileContext(nc) as tc, ExitStack() as ctx:
        singles = ctx.enter_context(tc.tile_pool(name="singles", bufs=1))
        s1_pool = ctx.enter_context(tc.tile_pool(name="s1", bufs=4))
        attn_pool = ctx.enter_context(tc.tile_pool(name="attn", bufs=2))
        r_pool = ctx.enter_context(tc.tile_pool(name="r", bufs=2))
        ao_pool = ctx.enter_context(tc.tile_pool(name="ao", bufs=2))
        out_pool = ctx.enter_context(tc.tile_pool(name="outp", bufs=3))

        ps_sc = ctx.enter_context(tc.tile_pool(name="ps_sc", bufs=2, space="PSUM"))
        ps_av = ctx.enter_context(tc.tile_pool(name="ps_av", bufs=2, space="PSUM"))
        ps_fin = ctx.enter_context(tc.tile_pool(name="ps_fin", bufs=2, space="PSUM"))

        # ---- SBUF singles ----
        wb = singles.tile([C, W16], F16, tag="wb")
        xh = singles.tile([C, HW], F16, tag="xh")
        kvh = singles.tile([KV + 1, L], F16, tag="kvh")  # v 0:64, k 64:80, ones 80
        ab = singles.tile([C, L], F16, tag="ab")
        ones65 = singles.tile([1, DV + 1], BF16, tag="ones")
        baux = singles.tile([C, WF32], F32, tag="baux")

        identh = wb[0:DV, 0:DV]
        w_qt17 = wb[DV : KV + 1, DV : DV + C]            # rows 64:81
        b_q64 = wb[DV : DV + D, DV + C : DV + C + 1]     # rows 64:80
        w_fin = wb[0 : DV + 1, DV + C + 1 : DV + 2 * C + 1]
        w_kv = wb[:, DV + 2 * C + 1 : DV + 2 * C + 1 + KV]
        bkbq = baux[:, 0:1]

        # ---- input DMAs: dispatch from both DMA-capable hardware queues
        # (sync, scalar/ACT) in parallel; piece 0 + weights first ----
        nc.sync.dma_start(out=xh[:, 0:QB], in_=xh_d[:, 0:QB])
        nc.scalar.dma_start(out=wb, in_=wb_d)
        nc.sync.dma_start(out=xh[:, QB : 2 * QB], in_=xh_d[:, QB : 2 * QB])
        nc.scalar.dma_start(out=xh[:, 2 * QB : 3 * QB], in_=xh_d[:, 2 * QB : 3 * QB])
        nc.sync.dma_start(out=xh[:, 3 * QB : 4 * QB], in_=xh_d[:, 3 * QB : 4 * QB])
        nc.scalar.dma_start(out=xh[:, 4 * QB : 5 * QB], in_=xh_d[:, 4 * QB : 5 * QB])
        nc.sync.dma_start(out=xh[:, 5 * QB : 6 * QB], in_=xh_d[:, 5 * QB : 6 * QB])
        nc.scalar.dma_start(out=xh[:, 6 * QB : 7 * QB], in_=xh_d[:, 6 * QB : 7 * QB])
        nc.sync.dma_start(out=xh[:, 7 * QB : 8 * QB], in_=xh_d[:, 7 * QB : 8 * QB])
        nc.scalar.dma_start(out=baux, in_=baux_d)

        nc.vector.memset(ones65, 1.0)
        # ones row (partition 80) for the A-matmul bias fold; whole-tile
        # memset (start partition must be 0), pool overwrites rows 0:80
        nc.gpsimd.memset(kvh, 1.0)

        attn0 = attn_pool.tile([KC, NCH * QB], BF16, tag="attn")
        vt_t = ps_av.tile([C, QB], F32, tag="av")  # 8x(128,64) vT chunks
        vt16 = vt_t.bitcast(F16)
        bqk_t = ps_av.tile([C, QB], F32, tag="av")  # cols 0:8 used

        def late_tail(c):
            # A_c matmul (bias folded via kvh ones row), ab copy, block-0
            # scores + 512-wide exp. sc0 tiles live in ps_fin so the proj
            # ring (ps_sc) never waits on block-0 exps.
            csl = slice(c * KC, (c + 1) * KC)
            a_ps = ps_fin.tile([C, QB], F32, tag="fin", name=f"a{c}")
            nc.tensor.matmul(
                a_ps[:, 0:KC], lhsT=w_qt17, rhs=kvh[DV : KV + 1, csl],
                start=True, stop=True, tile_position=(DV, 0),
            )
            nc.vector.tensor_copy(ab[:, csl], a_ps[:, 0:KC])
            sc0c = ps_fin.tile([KC, QB], F32, tag="fin", name=f"sc0_{c}")
            nc.tensor.matmul(
                sc0c[:, :],
                lhsT=ab[:, csl],
                rhs=xh[:, 0:QB],
                start=True,
                stop=True,
            )
            nc.scalar.activation(
                attn0[:, c * QB : (c + 1) * QB],
                sc0c[:, :],
                mybir.ActivationFunctionType.Exp,
            )
            nc.tensor.matmul(
                bqk_t[:, c : c + 1], lhsT=kvh[DV : DV + D, csl], rhs=b_q64,
                start=True, stop=True, tile_position=(DV, 0),
            )
            nc.tensor.transpose(
                vt16[:, c * DV : (c + 1) * DV], kvh[0:DV, csl], identh
            )

        # ---- prologue: kv proj + pool chain, block-0 scores interleaved ----
        proj = None
        for c in range(NCH):
            j = c % 2
            if j == 0:
                proj = ps_sc.tile([KC, 2 * QB], F32, tag="sc", name=f"proj{c}")
            sl = slice(c * QB, (c + 1) * QB)
            nc.tensor.matmul(
                proj[:KV, j * QB : (j + 1) * QB],
                lhsT=w_kv,
                rhs=xh[:, sl],
                start=True,
                stop=True,
            )
            csl = slice(c * KC, (c + 1) * KC)
            # 2x2 maxpool via two DVE reduces (w-pairs from psum, then h-pairs)
            pv = proj[:KV, j * QB : (j + 1) * QB].rearrange(
                "p (w two) -> p w two", two=2
            )
            s1 = s1_pool.tile([KV, 256], F32, tag="s1")
            nc.vector.tensor_reduce(
                s1[:, :], pv, mybir.AxisListType.X, mybir.AluOpType.max
            )
            sv = s1.rearrange("p (h two w) -> p h w two", h=4, two=2)
            nc.vector.tensor_reduce(
                kvh[:KV, csl], sv, mybir.AxisListType.X, mybir.AluOpType.max
            )
            if c >= 1:
                late_tail(c - 1)
        late_tail(NCH - 1)
        # (defer_kv_aux is emitted right below, after its definition)

        ebqk = singles.tile([KC, NCH], F32, tag="ebqk")
        vaug = singles.tile([KC, NCH * (DV + 1)], BF16, tag="vaug")

        # ebqk + vaug assembly (vt/bqk matmuls already ran inside late_tail)
        nc.scalar.activation(
            ebqk[:, :], bqk_t[:, 0:NCH],
            mybir.ActivationFunctionType.Exp, bias=bkbq,
        )
        for c in range(NCH):
            base = c * (DV + 1)
            nc.vector.tensor_scalar_mul(
                vaug[:, base : base + DV],
                vt16[:, c * DV : (c + 1) * DV],
                ebqk[:, c : c + 1],
            )
            nc.vector.tensor_copy(
                vaug[:, base + DV : base + DV + 1], ebqk[:, c : c + 1]
            )

        # ---- main loop: 4-deep software pipeline (block 0 prefilled) ----
        # iter i: PE [sc(i) x8 | av(i-1) x8 | fin(i-3)]
        #         ACT [exp(i) tiles 0-2], DVE [schraudolph exp tile 3,
        #              dn+recip(i-1), ao-mul(i-2), residual-add(i-3)]
        #         DMA [r bounce (i-2), out (i-3)]
        attn_t, av_t, r_t, R65s_t, ao_t = {}, {}, {}, {}, {}
        attn_t[0] = attn0

        for i in range(1, NB + 3):
            b_sc = i          # scores + exp
            b_av = i - 1      # av accumulation + recip + bounce dispatch
            b_r = i - 2       # ao mul + fin + residual + store

            if b_sc < NB:
                qsl = slice(b_sc * QB, (b_sc + 1) * QB)
                attn = attn_pool.tile([KC, NCH * QB], BF16, tag="attn")
                attn_t[b_sc] = attn
                attn16 = attn.bitcast(I16)
                for t in range(4):
                    sc = ps_sc.tile([KC, 2 * QB], F32, tag="sc")
                    for j in range(2):
                        cc = 2 * t + j
                        nc.tensor.matmul(
                            sc[:, j * QB : (j + 1) * QB],
                            lhsT=ab[:, cc * KC : (cc + 1) * KC],
                            rhs=xh[:, qsl],
                            start=True,
                            stop=True,
                        )
                    # interleave av MMs of previous block between score tiles
                    if t == 1 and 0 <= b_av < NB:
                        _av_mms(nc, ps_av, av_t, vaug, attn_t, b_av, 0, 4)
                    if t == 2 and 0 <= b_av < NB:
                        _av_mms(nc, ps_av, av_t, vaug, attn_t, b_av, 4, 8)
                    if t > 0:
                        nc.scalar.activation(
                            attn[:, t * 2 * QB : (t + 1) * 2 * QB],
                            sc[:, :],
                            mybir.ActivationFunctionType.Exp,
                        )
                    else:
                        # Schraudolph exp on DVE: bf16 bits via int16 affine
                        nc.vector.tensor_scalar(
                            attn16[:, t * 2 * QB : (t + 1) * 2 * QB],
                            sc[:, :],
                            SCH_A,
                            SCH_B,
                            mybir.AluOpType.mult,
                            mybir.AluOpType.add,
                        )
                if b_sc == NB - 1:
                    # last block: start av(7) chunks 0-3 as soon as its first
                    # exps land (rest in the next iteration)
                    _av_mms(nc, ps_av, av_t, vaug, attn_t, b_sc, 0, 4)
            elif 0 <= b_av < NB:
                c0 = 4 if b_av == NB - 1 else 0
                _av_mms(nc, ps_av, av_t, vaug, attn_t, b_av, c0, 8)

            if 0 <= b_av < NB:
                # recip of denominators as soon as av(b_av) stops
                # (custom-DVE recip must read SBUF: stage the psum row first)
                dn = r_pool.tile([1, QB], F32, tag="dn", name=f"dn{b_av}")
                r = r_pool.tile([1, QB], F32, tag="r", name=f"r{b_av}")
                nh = 2 if b_av >= NB - 2 else 1
                for h in range(nh):
                    hs = slice(h * QB // nh, (h + 1) * QB // nh)
                    if nh == 2:  # tail: stage on the by-then-idle ACT
                        nc.scalar.copy(dn[:, hs], av_t[b_av][DV : DV + 1, hs])
                    else:
                        nc.vector.tensor_copy(dn[:, hs], av_t[b_av][DV : DV + 1, hs])
                    nc.vector.reciprocal_approx_fast(r[:, hs], dn[:, hs])
                r_t[b_av] = r
                if b_av < NB - 2:
                    # broadcast r over 65 partitions via PE ones-matmul, a
                    # full iteration before ao-mul needs it (no DMA bounce:
                    # the sync queue then only carries output stores)
                    R65s = r_pool.tile(
                        [DV + 1, QB], F32, tag="R65s", name=f"R65s{b_av}"
                    )
                    rb = r_pool.tile([1, QB], BF16, tag="rb", name=f"rb{b_av}")
                    R65p = ps_fin.tile([C, QB], F32, tag="fin")
                    nc.scalar.copy(rb[:, :], r[:, :])
                    nc.tensor.matmul(
                        R65p[0 : DV + 1, :], lhsT=ones65, rhs=rb[:, :],
                        start=True, stop=True,
                    )
                    nc.scalar.copy(R65s[:, :], R65p[0 : DV + 1, :])
                    R65s_t[b_av] = R65s

            if 0 <= b_r < NB:
                # ao + fin + residual + store all in one stage: the bounce
                # was dispatched a full iteration ago so R65s has landed
                if b_r >= NB - 2:
                    # tail blocks: low-latency PE ones-matmul broadcast,
                    # 256-wide halves, copies on the by-then-idle ACT
                    R65s = r_pool.tile(
                        [DV + 1, QB], F32, tag="R65s", name=f"R65s{b_r}"
                    )
                    rb = r_pool.tile([1, QB], BF16, tag="rb", name=f"rb{b_r}")
                    R65p = ps_fin.tile([C, QB], F32, tag="fin")
                    for h in range(2):
                        hs = slice(h * 256, (h + 1) * 256)
                        nc.scalar.copy(rb[:, hs], r_t[b_r][:, hs])
                        nc.tensor.matmul(
                            R65p[0 : DV + 1, hs], lhsT=ones65, rhs=rb[:, hs],
                            start=True, stop=True,
                        )
                        nc.scalar.copy(R65s[:, hs], R65p[0 : DV + 1, hs])
                    R65s_t[b_r] = R65s
                ao = ao_pool.tile([DV + 1, QB], F16, tag="ao")
                fin = ps_fin.tile([C, QB], F32, tag="fin")
                o = out_pool.tile([C, QB], F32, tag="o")
                if b_r >= NB - 2:
                    for h in range(2):
                        hs = slice(h * 256, (h + 1) * 256)
                        qh = slice(b_r * QB + h * 256, b_r * QB + (h + 1) * 256)
                        nc.vector.tensor_mul(
                            ao[:, hs], av_t[b_r][:, hs], R65s_t[b_r][:, hs]
                        )
                        nc.tensor.matmul(
                            fin[:, hs], lhsT=w_fin, rhs=ao[:, hs],
                            start=True, stop=True,
                        )
                        nc.vector.tensor_add(o[:, hs], fin[:, hs], xh[:, qh])
                        nc.sync.dma_start(out=out_d[:, qh], in_=o[:, hs])
                else:
                    qsl = slice(b_r * QB, (b_r + 1) * QB)
                    nc.vector.tensor_mul(
                        ao[:, :], av_t[b_r][:, :], R65s_t[b_r][:, :]
                    )
                    nc.tensor.matmul(
                        fin[:, :], lhsT=w_fin, rhs=ao[:, :],
                        start=True, stop=True,
                    )
                    nc.vector.tensor_add(o[:, :], fin[:, :], xh[:, qsl])
                    nc.sync.dma_start(out=out_d[:, qsl], in_=o[:, :])

    nc.compile()
    return nc


def _av_mms(nc, ps_av, av_t, vaug, attn_t, b, c0, c1):
    if b not in av_t:
        av_t[b] = ps_av.tile([DV + 1, QB], F32, tag="av", name=f"av{b}")
    av = av_t[b]
    attn = attn_t[b]
    for c in range(c0, c1):
        base = c * (DV + 1)
        nc.tensor.matmul(
            av[:, :],
            lhsT=vaug[:, base : base + DV + 1],
            rhs=attn[:, c * QB : (c + 1) * QB],
            start=(c == 0),
            stop=(c == NCH - 1),
        )


def prep_weights(Wq, bq, Wk, bk, Wv, bv, Wo, bo, gamma):
    g = np.float32(np.asarray(gamma))
    Wq, Wk, Wv, Wo = (np.asarray(a, np.float32) for a in (Wq, Wk, Wv, Wo))
    bq_, bk_, bv_, bo_ = (np.asarray(a, np.float32) for a in (bq, bk, bv, bo))
    wb = np.zeros((C, W16), np.float16)
    wb[0:DV, 0:DV] = np.eye(DV, dtype=np.float16)
    wb[DV : DV + D, DV : DV + C] = Wq.T.astype(np.float16)
    wb[KV, DV : DV + C] = (Wq @ bk_).astype(np.float16)  # bias row (ones fold)
    wb[DV : DV + D, DV + C] = bq_.astype(np.float16)
    wb[0:DV, DV + C + 1 : DV + 2 * C + 1] = (g * Wo).astype(np.float16)
    # bo' = bo + Wo.T bv  (v-bias folded host-side)
    wb[DV, DV + C + 1 : DV + 2 * C + 1] = (g * (bo_ + Wo.T @ bv_)).astype(
        np.float16
    )
    wb[:, DV + 2 * C + 1 : DV + 2 * C + 1 + DV] = Wv.astype(np.float16)
    wb[:, DV + 2 * C + 1 + DV : DV + 2 * C + 1 + KV] = Wk.astype(np.float16)
    baux = np.zeros((C, WF32), np.float32)
    baux[:, 0] = np.float32(bk_ @ bq_)  # bqk scalar bias
    return np.ascontiguousarray(wb), np.ascontiguousarray(baux)


_NC_CACHE = {}


def kernel(x, Wq, bq, Wk, bk, Wv, bv, Wo, bo, gamma):
    x = np.asarray(x, dtype=np.float32)
    N = x.shape[0]
    assert x.shape == (N, C, 64, 64) and N == NCORES
    wb, baux = prep_weights(Wq, bq, Wk, bk, Wv, bv, Wo, bo, gamma)

    if "nc" not in _NC_CACHE:
        _NC_CACHE["nc"] = build_kernel()
    nc = _NC_CACHE["nc"]

    in_maps = []
    for i in range(N):
        in_maps.append(
            {
                "xh": np.ascontiguousarray(x[i].reshape(C, HW).astype(np.float16)),
                "wb": wb,
                "baux": baux,
            }
        )
    res = bass_utils.run_bass_kernel_spmd(nc, in_maps, core_ids=list(range(N)))
    out = np.stack([res.results[i]["out"].reshape(C, 64, 64) for i in range(N)])
    return out.astype(np.float32)


if __name__ == "__main__":
    print("built", build_kernel())


# revision 36
# speedup vs baseline: 1.1133x; 1.0323x over previous
"""NonLocalAttention2D Trainium2 kernel (v21).

Data-parallel over batch N=8: one image per NeuronCore.

Per-core math (xh: (C=128, HW=4096) fp16, cast from fp32 x on the host so
only 1MB of input DMA gates the prologue and no on-chip casts are needed;
the residual uses fp16 x, error ~3e-4 of output scale):
  kv   = [Wv|Wk].T @ xh              (80, 4096)  PE fp16 (v rows 0:64, k 64:80)
  pool = maxpool2x2(kv)              (80, 1024)  DVE: two strided max-reduces
  A    = [Wq.T; Wq@bk].T @ [K; 1]    (128, 1024) PE fp16 (bias folded via ones
                                     row 80 of kvh), DVE copy -> ab fp16
  bqk  = k.T @ bq, ebqk = exp(.)     (128, 8)    PE + ACT (bias bk.bq)
  vaugT= [vT*ebqk | ebqk]            (128, 8*65) PE transpose f16 + DVE -> bf16
  s_cb = ab_c.T @ xh_b               (128k,512q) PE fp16 -> psum
  attn = exp(s): tiles 1-3 ACT Exp; tile 0 DVE Schraudolph
         (int16(s*128/ln2 + 16252.5) bitcast bf16, ~2% rel err that cancels
         in the softmax normalization; GpSimd cannot read PSUM so only
         ACT/DVE can exp). Tile 0 on DVE keeps the attn tile's last writer
         on ACT, so av never waits on the DVE queue.
  av   = vaugT.T @ attn  (accum 8c)  (65, 512)   PE bf16; row 64 = denom
  r    = recip_approx_fast(denom)    (1, 512)    DVE (input staged to SBUF)
  R65  = broadcast r over 65 parts:  PE ones-matmul (rb bf16 via ACT copy,
         psum -> R65s via ACT copy), issued one iteration before use. A
         DRAM-bounce broadcast serializes the sync DMA queue - avoid.
  ao   = av * R65 -> fp16            (65, 512)   DVE
  fin  = [g*Wo; g*bo'].T @ ao        (128, 512)  PE fp16, same stage as ao
  out  = fin + xh_b                  (128, 512)  DVE -> DMA out

Pipeline: 3-deep (scores/exp | av+recip+broadcast | ao+fin+residual+store),
block-0 scores/exps AND the vt-transpose/bqk matmuls interleaved into the
kv/pool prologue chunk by chunk (sc0 tiles in ps_fin so the proj ring in
ps_sc never waits on exps; vt/bqk in ps_av so nothing waits on block-0
exps; ebqk+vaug assembled right after the prologue). Input DMAs dispatch
from both hardware DMA queues (sync + scalar). Last two blocks run their
chain in 256-col halves with the staging copies on the by-then-idle ACT.

PSUM budget (8 banks): ps_sc 2x[128,1024] scores ring = 4, ps_av
2x[128,512] av/vt/bqk = 2, ps_fin 2x[128,512] fin/sc0/a_ps/R65p = 2.
"""

import sys

if "/opt/trn_rl_repo" not in sys.path:
    sys.path.insert(0, "/opt/trn_rl_repo")

import numpy as np

import concourse.bacc as bacc
import concourse.bass as bass
import concourse.tile as tile
from concourse import bass_utils, mybir

F32 = mybir.dt.float32
F16 = mybir.dt.float16
BF16 = mybir.dt.bfloat16
I16 = mybir.dt.int16

C = 128          # channels
HW = 4096        # 64*64 pixels
L = 1024         # pooled keys (32*32)
D = 16           # attn dim
DV = 64          # value dim
KV = 80          # kv projection out width (v rows 0:64, k rows 64:80)
QB = 512         # q-block size
NB = HW // QB    # 8 q blocks
KC = 128         # keys per chunk
NCH = L // KC    # 8 key chunks
NCORES = 8
# wb: ident64 | wqt17 (rows 64:81) | bq (rows 64:80) | wfin | wkv
W16 = DV + C + 1 + C + KV
WF32 = 1                   # bkbq

# Schraudolph exp -> bf16 bits: bits = trunc(s * 2^7/ln2 + (127*2^7 - 4 + 0.5))
SCH_A = 128.0 / 0.6931471805599453
SCH_B = 16252.5


def build_kernel():
    nc = bacc.Bacc("TRN2", target_bir_lowering=False, debug=False)

    xh_d = nc.dram_tensor("xh", (C, HW), F16, kind="ExternalInput").ap()
    wb_d = nc.dram_tensor("wb", (C, W16), F16, kind="ExternalInput").ap()
    baux_d = nc.dram_tensor("baux", (C, WF32), F32, kind="ExternalInput").ap()
    out_d = nc.dram_tensor("out", (C, HW), F32, kind="ExternalOutput").ap()

    from contextlib import ExitStack

    with tile.TileContext(nc) as tc, ExitStack() as ctx:
        singles = ctx.enter_context(tc.tile_pool(name="singles", bufs=1))
        s1_pool = ctx.enter_context(tc.tile_pool(name="s1", bufs=4))
        attn_pool = ctx.enter_context(tc.tile_pool(name="attn", bufs=2))
        r_pool = ctx.enter_context(tc.tile_pool(name="r", bufs=2))
        ao_pool = ctx.enter_context(tc.tile_pool(name="ao", bufs=2))
        out_pool = ctx.enter_context(tc.tile_pool(name="outp", bufs=3))

        ps_sc = ctx.enter_context(tc.tile_pool(name="ps_sc", bufs=2, space="PSUM"))
        ps_av = ctx.enter_context(tc.tile_pool(name="ps_av", bufs=2, space="PSUM"))
        ps_fin = ctx.enter_context(tc.tile_pool(name="ps_fin", bufs=2, space="PSUM"))

        # ---- SBUF singles ----
        wb = singles.tile([C, W16], F16, tag="wb")
        xh = singles.tile([C, HW], F16, tag="xh")
        kvh = singles.tile([KV + 1, L], F16, tag="kvh")  # v 0:64, k 64:80, ones 80
        ab = singles.tile([C, L], F16, tag="ab")
        ones65 = singles.tile([1, DV + 1], BF16, tag="ones")
        baux = singles.tile([C, WF32], F32, tag="baux")

        identh = wb[0:DV, 0:DV]
        w_qt17 = wb[DV : KV + 1, DV : DV + C]            # rows 64:81
        b_q64 = wb[DV : DV + D, DV + C : DV + C + 1]     # rows 64:80
        w_fin = wb[0 : DV + 1, DV + C + 1 : DV + 2 * C + 1]
        w_kv = wb[:, DV + 2 * C + 1 : DV + 2 * C + 1 + KV]
        bkbq = baux[:, 0:1]

        # ---- input DMAs: dispatch from both DMA-capable hardware queues
        # (sync, scalar/ACT) in parallel; piece 0 + weights first ----
        nc.sync.dma_start(out=xh[:, 0:QB], in_=xh_d[:, 0:QB])
        nc.scalar.dma_start(out=wb, in_=wb_d)
        nc.sync.dma_start(out=xh[:, QB : 2 * QB], in_=xh_d[:, QB : 2 * QB])
        nc.scalar.dma_start(out=xh[:, 2 * QB : 3 * QB], in_=xh_d[:, 2 * QB : 3 * QB])
        nc.sync.dma_start(out=xh[:, 3 * QB : 4 * QB], in_=xh_d[:, 3 * QB : 4 * QB])
        nc.scalar.dma_start(out=xh[:, 4 * QB : 5 * QB], in_=xh_d[:, 4 * QB : 5 * QB])
        nc.sync.dma_start(out=xh[:, 5 * QB : 6 * QB], in_=xh_d[:, 5 * QB : 6 * QB])
        nc.scalar.dma_start(out=xh[:, 6 * QB : 7 * QB], in_=xh_d[:, 6 * QB : 7 * QB])
        nc.sync.dma_start(out=xh[:, 7 * QB : 8 * QB], in_=xh_d[:, 7 * QB : 8 * QB])
        nc.scalar.dma_start(out=baux, in_=baux_d)

        nc.vector.memset(ones65, 1.0)
        # ones row (partition 80) for the A-matmul bias fold; whole-tile
        # memset (start partition must be 0), pool overwrites rows 0:80
        nc.gpsimd.memset(kvh, 1.0)

        attn0 = attn_pool.tile([KC, NCH * QB], BF16, tag="attn")
        vt_t = ps_av.tile([C, QB], F32, tag="av")  # 8x(128,64) vT chunks
        vt16 = vt_t.bitcast(F16)
        bqk_t = ps_av.tile([C, QB], F32, tag="av")  # cols 0:8 used

        def late_tail(c):
            # A_c matmul (bias folded via kvh ones row), ab copy, block-0
            # scores + 512-wide exp. sc0 tiles live in ps_fin so the proj
            # ring (ps_sc) never waits on block-0 exps.
            csl = slice(c * KC, (c + 1) * KC)
            a_ps = ps_fin.tile([C, QB], F32, tag="fin", name=f"a{c}")
            nc.tensor.matmul(
                a_ps[:, 0:KC], lhsT=w_qt17, rhs=kvh[DV : KV + 1, csl],
                start=True, stop=True, tile_position=(DV, 0),
            )
            nc.vector.tensor_copy(ab[:, csl], a_ps[:, 0:KC])
            sc0c = ps_fin.tile([KC, QB], F32, tag="fin", name=f"sc0_{c}")
            nc.tensor.matmul(
                sc0c[:, :],
                lhsT=ab[:, csl],
                rhs=xh[:, 0:QB],
                start=True,
                stop=True,
            )
            nc.scalar.activation(
                attn0[:, c * QB : (c + 1) * QB],
                sc0c[:, :],
                mybir.ActivationFunctionType.Exp,
            )
            nc.tensor.matmul(
                bqk_t[:, c : c + 1], lhsT=kvh[DV : DV + D, csl], rhs=b_q64,
                start=True, stop=True, tile_position=(DV, 0),
            )
            nc.tensor.transpose(
                vt16[:, c * DV : (c + 1) * DV], kvh[0:DV, csl], identh
            )

        # ---- prologue: kv proj + pool chain, block-0 scores interleaved ----
        proj = None
        for c in range(NCH):
            j = c % 2
            if j == 0:
                proj = ps_sc.tile([KC, 2 * QB], F32, tag="sc", name=f"proj{c}")
            sl = slice(c * QB, (c + 1) * QB)
            nc.tensor.matmul(
                proj[:KV, j * QB : (j + 1) * QB],
                lhsT=w_kv,
                rhs=xh[:, sl],
                start=True,
                stop=True,
            )
            csl = slice(c * KC, (c + 1) * KC)
            # 2x2 maxpool via two DVE reduces (w-pairs from psum, then h-pairs)
            pv = proj[:KV, j * QB : (j + 1) * QB].rearrange(
                "p (w two) -> p w two", two=2
            )
            s1 = s1_pool.tile([KV, 256], F32, tag="s1")
            nc.vector.tensor_reduce(
                s1[:, :], pv, mybir.AxisListType.X, mybir.AluOpType.max
            )
            sv = s1.rearrange("p (h two w) -> p h w two", h=4, two=2)
            nc.vector.tensor_reduce(
                kvh[:KV, csl], sv, mybir.AxisListType.X, mybir.AluOpType.max
            )
            if c >= 1:
                late_tail(c - 1)
        late_tail(NCH - 1)
        # (defer_kv_aux is emitted right below, after its definition)

        ebqk = singles.tile([KC, NCH], F32, tag="ebqk")
        vaug = singles.tile([KC, NCH * (DV + 1)], BF16, tag="vaug")

        # ebqk + vaug assembly (vt/bqk matmuls already ran inside late_tail)
        nc.scalar.activation(
            ebqk[:, :], bqk_t[:, 0:NCH],
            mybir.ActivationFunctionType.Exp, bias=bkbq,
        )
        for c in range(NCH):
            base = c * (DV + 1)
            nc.vector.tensor_scalar_mul(
                vaug[:, base : base + DV],
                vt16[:, c * DV : (c + 1) * DV],
                ebqk[:, c : c + 1],
            )
            nc.vector.tensor_copy(
                vaug[:, base + DV : base + DV + 1], ebqk[:, c : c + 1]
            )

        # ---- main loop: 4-deep software pipeline (block 0 prefilled) ----
        # iter i: PE [sc(i) x8 | av(i-1) x8 | fin(i-3)]
        #         ACT [exp(i) tiles 0-2], DVE [schraudolph exp tile 3,
        #              dn+recip(i-1), ao-mul(i-2), residual-add(i-3)]
        #         DMA [r bounce (i-2), out (i-3)]
        attn_t, av_t, r_t, R65s_t, ao_t = {}, {}, {}, {}, {}
        attn_t[0] = attn0

        for i in range(1, NB + 3):
            b_sc = i          # scores + exp
            b_av = i - 1      # av accumulation + recip + bounce dispatch
            b_r = i - 2       # ao mul + fin + residual + store

            if b_sc < NB:
                qsl = slice(b_sc * QB, (b_sc + 1) * QB)
                attn = attn_pool.tile([KC, NCH * QB], BF16, tag="attn")
                attn_t[b_sc] = attn
                attn16 = attn.bitcast(I16)
                for t in range(4):
                    sc = ps_sc.tile([KC, 2 * QB], F32, tag="sc")
                    for j in range(2):
                        cc = 2 * t + j
                        nc.tensor.matmul(
                            sc[:, j * QB : (j + 1) * QB],
                            lhsT=ab[:, cc * KC : (cc + 1) * KC],
                            rhs=xh[:, qsl],
                            start=True,
                            stop=True,
                        )
                    # interleave av MMs of previous block between score tiles
                    if t == 1 and 0 <= b_av < NB:
                        _av_mms(nc, ps_av, av_t, vaug, attn_t, b_av, 0, 4)
                    if t == 2 and 0 <= b_av < NB:
                        _av_mms(nc, ps_av, av_t, vaug, attn_t, b_av, 4, 8)
                    if t > 0:
                        nc.scalar.activation(
                            attn[:, t * 2 * QB : (t + 1) * 2 * QB],
                            sc[:, :],
                            mybir.ActivationFunctionType.Exp,
                        )
                    else:
                        # Schraudolph exp on DVE: bf16 bits via int16 affine
                        nc.vector.tensor_scalar(
                            attn16[:, t * 2 * QB : (t + 1) * 2 * QB],
                            sc[:, :],
                            SCH_A,
                            SCH_B,
                            mybir.AluOpType.mult,
                            mybir.AluOpType.add,
                        )
                if b_sc == NB - 1:
                    # last block: start av(7) chunks 0-3 as soon as its first
                    # exps land (rest in the next iteration)
                    _av_mms(nc, ps_av, av_t, vaug, attn_t, b_sc, 0, 4)
            elif 0 <= b_av < NB:
                c0 = 4 if b_av == NB - 1 else 0
                _av_mms(nc, ps_av, av_t, vaug, attn_t, b_av, c0, 8)

            if 0 <= b_av < NB:
                # recip of denominators as soon as av(b_av) stops
                # (custom-DVE recip must read SBUF: stage the psum row first)
                dn = r_pool.tile([1, QB], F32, tag="dn", name=f"dn{b_av}")
                r = r_pool.tile([1, QB], F32, tag="r", name=f"r{b_av}")
                nh = 2 if b_av >= NB - 2 else 1
                for h in range(nh):
                    hs = slice(h * QB // nh, (h + 1) * QB // nh)
                    if nh == 2:  # tail: stage on the by-then-idle ACT
                        nc.scalar.copy(dn[:, hs], av_t[b_av][DV : DV + 1, hs])
                    else:
                        nc.vector.tensor_copy(dn[:, hs], av_t[b_av][DV : DV + 1, hs])
                    nc.vector.reciprocal_approx_fast(r[:, hs], dn[:, hs])
                r_t[b_av] = r
                if b_av < NB - 2:
                    # broadcast r over 65 partitions via PE ones-matmul, a
                    # full iteration before ao-mul needs it (no DMA bounce:
                    # the sync queue then only carries output stores)
                    R65s = r_pool.tile(
                        [DV + 1, QB], F32, tag="R65s", name=f"R65s{b_av}"
                    )
                    rb = r_pool.tile([1, QB], BF16, tag="rb", name=f"rb{b_av}")
                    R65p = ps_fin.tile([C, QB], F32, tag="fin")
                    nc.scalar.copy(rb[:, :], r[:, :])
                    nc.tensor.matmul(
                        R65p[0 : DV + 1, :], lhsT=ones65, rhs=rb[:, :],
                        start=True, stop=True,
                    )
                    nc.scalar.copy(R65s[:, :], R65p[0 : DV + 1, :])
                    R65s_t[b_av] = R65s

            if 0 <= b_r < NB:
                # ao + fin + residual + store all in one stage: the bounce
                # was dispatched a full iteration ago so R65s has landed
                if b_r >= NB - 2:
                    # tail blocks: low-latency PE ones-matmul broadcast,
                    # 256-wide halves, copies on the by-then-idle ACT
                    R65s = r_pool.tile(
                        [DV + 1, QB], F32, tag="R65s", name=f"R65s{b_r}"
                    )
                    rb = r_pool.tile([1, QB], BF16, tag="rb", name=f"rb{b_r}")
                    R65p = ps_fin.tile([C, QB], F32, tag="fin")
                    for h in range(2):
                        hs = slice(h * 256, (h + 1) * 256)
                        nc.scalar.copy(rb[:, hs], r_t[b_r][:, hs])
                        nc.tensor.matmul(
                            R65p[0 : DV + 1, hs], lhsT=ones65, rhs=rb[:, hs],
                            start=True, stop=True,
                        )
                        nc.scalar.copy(R65s[:, hs], R65p[0 : DV + 1, hs])
                    R65s_t[b_r] = R65s
                ao = ao_pool.tile([DV + 1, QB], F16, tag="ao")
                fin = ps_fin.tile([C, QB], F32, tag="fin")
                o = out_pool.tile([C, QB], F32, tag="o")
                if b_r >= NB - 2:
                    for h in range(2):
                        hs = slice(h * 256, (h + 1) * 256)
                        qh = slice(b_r * QB + h * 256, b_r * QB + (h + 1) * 256)
                        nc.vector.tensor_mul(
                            ao[:, hs], av_t[b_r][:, hs], R65s_t[b_r][:, hs]
                        )
                        nc.tensor.matmul(
                            fin[:, hs], lhsT=w_fin, rhs=ao[:, hs],
                            start=True, stop=True,
                        )
                        nc.vector.tensor_add(o[:, hs], fin[:, hs], xh[:, qh])
                        nc.sync.dma_start(out=out_d[:, qh], in_=o[:, hs])
                else:
                    qsl = slice(b_r * QB, (b_r + 1) * QB)
                    nc.vector.tensor_mul(
                        ao[:, :], av_t[b_r][:, :], R65s_t[b_r][:, :]
                    )
                    nc.tensor.matmul(
                        fin[:, :], lhsT=w_fin, rhs=ao[:, :],
                        start=True, stop=True,
                    )
                    nc.vector.tensor_add(o[:, :], fin[:, :], xh[:, qsl])
                    nc.sync.dma_start(out=out_d[:, qsl], in_=o[:, :])

    nc.compile()
    return nc


def _av_mms(nc, ps_av, av_t, vaug, attn_t, b, c0, c1):
    if b not in av_t:
        av_t[b] = ps_av.tile([DV + 1, QB], F32, tag="av", name=f"av{b}")
    av = av_t[b]
    attn = attn_t[b]
    for c in range(c0, c1):
        base = c * (DV + 1)
        nc.tensor.matmul(
            av[:, :],
            lhsT=vaug[:, base : base + DV + 1],
            rhs=attn[:, c * QB : (c + 1) * QB],
            start=(c == 0),
            stop=(c == NCH - 1),
        )


def prep_weights(Wq, bq, Wk, bk, Wv, bv, Wo, bo, gamma):
    g = np.float32(np.asarray(gamma))
    Wq, Wk, Wv, Wo = (np.asarray(a, np.float32) for a in (Wq, Wk, Wv, Wo))
    bq_, bk_, bv_, bo_ = (np.asarray(a, np.float32) for a in (bq, bk, bv, bo))
    wb = np.zeros((C, W16), np.float16)
    wb[0:DV, 0:DV] = np.eye(DV, dtype=np.float16)
    wb[DV : DV + D, DV : DV + C] = Wq.T.astype(np.float16)
    wb[KV, DV : DV + C] = (Wq @ bk_).astype(np.float16)  # bias row (ones fold)
    wb[DV : DV + D, DV + C] = bq_.astype(np.float16)
    wb[0:DV, DV + C + 1 : DV + 2 * C + 1] = (g * Wo).astype(np.float16)
    # bo' = bo + Wo.T bv  (v-bias folded host-side)
    wb[DV, DV + C + 1 : DV + 2 * C + 1] = (g * (bo_ + Wo.T @ bv_)).astype(
        np.float16
    )
    wb[:, DV + 2 * C + 1 : DV + 2 * C + 1 + DV] = Wv.astype(np.float16)
    wb[:, DV + 2 * C + 1 + DV : DV + 2 * C + 1 + KV] = Wk.astype(np.float16)
    baux = np.zeros((C, WF32), np.float32)
    baux[:, 0] = np.float32(bk_ @ bq_)  # bqk scalar bias
    return np.ascontiguousarray(wb), np.ascontiguousarray(baux)


_NC_CACHE = {}


def kernel(x, Wq, bq, Wk, bk, Wv, bv, Wo, bo, gamma):
    x = np.asarray(x, dtype=np.float32)
    N = x.shape[0]
    assert x.shape == (N, C, 64, 64) and N == NCORES
    wb, baux = prep_weights(Wq, bq, Wk, bk, Wv, bv, Wo, bo, gamma)

    if "nc" not in _NC_CACHE:
        _NC_CACHE["nc"] = build_kernel()
    nc = _NC_CACHE["nc"]

    in_maps = []
    for i in range(N):
        in_maps.append(
            {
                "xh": np.ascontiguousarray(x[i].reshape(C, HW).astype(np.float16)),
                "wb": wb,
                "baux": baux,
            }
        )
    res = bass_utils.run_bass_kernel_spmd(nc, in_maps, core_ids=list(range(N)))
    out = np.stack([res.results[i]["out"].reshape(C, 64, 64) for i in range(N)])
    return out.astype(np.float32)


if __name__ == "__main__":
    print("built", build_kernel())


# revision 37
# speedup vs baseline: 1.1271x; 1.0124x over previous
"""NonLocalAttention2D Trainium2 kernel (v21).

Data-parallel over batch N=8: one image per NeuronCore.

Per-core math (xh: (C=128, HW=4096) fp16, cast from fp32 x on the host so
only 1MB of input DMA gates the prologue and no on-chip casts are needed;
the residual uses fp16 x, error ~3e-4 of output scale):
  kv   = [Wv|Wk].T @ xh              (80, 4096)  PE fp16 (v rows 0:64, k 64:80)
  pool = maxpool2x2(kv)              (80, 1024)  DVE: two strided max-reduces
  A    = [Wq.T; Wq@bk].T @ [K; 1]    (128, 1024) PE fp16 (bias folded via ones
                                     row 80 of kvh), DVE copy -> ab fp16
  bqk  = k.T @ bq, ebqk = exp(.)     (128, 8)    PE + ACT (bias bk.bq)
  vaugT= [vT*ebqk | ebqk]            (128, 8*65) PE transpose f16 + DVE -> bf16
  s_cb = ab_c.T @ xh_b               (128k,512q) PE fp16 -> psum
  attn = exp(s): tiles 1-3 ACT Exp; tile 0 DVE Schraudolph
         (int16(s*128/ln2 + 16252.5) bitcast bf16, ~2% rel err that cancels
         in the softmax normalization; GpSimd cannot read PSUM so only
         ACT/DVE can exp). Tile 0 on DVE keeps the attn tile's last writer
         on ACT, so av never waits on the DVE queue.
  av   = vaugT.T @ attn  (accum 8c)  (65, 512)   PE bf16; row 64 = denom
  r    = recip_approx_fast(denom)    (1, 512)    DVE (input staged to SBUF)
  R65  = broadcast r over 65 parts:  PE ones-matmul (rb bf16 via ACT copy,
         psum -> R65s via ACT copy), issued one iteration before use. A
         DRAM-bounce broadcast serializes the sync DMA queue - avoid.
  ao   = av * R65 -> fp16            (65, 512)   DVE
  fin  = [g*Wo; g*bo'].T @ ao        (128, 512)  PE fp16, same stage as ao
  out  = fin + xh_b                  (128, 512)  DVE -> DMA out

Pipeline: 3-deep (scores/exp | av+recip+broadcast | ao+fin+residual+store),
block-0 scores/exps AND the vt-transpose/bqk matmuls interleaved into the
kv/pool prologue chunk by chunk (sc0 tiles in ps_fin so the proj ring in
ps_sc never waits on exps; vt/bqk in ps_av so nothing waits on block-0
exps; ebqk+vaug assembled right after the prologue). Input DMAs dispatch
from both hardware DMA queues (sync + scalar). Last two blocks run their
chain in 256-col halves with the staging copies on the by-then-idle ACT.

PSUM budget (8 banks): ps_sc 2x[128,1024] scores ring = 4, ps_av
2x[128,512] av/vt/bqk = 2, ps_fin 2x[128,512] fin/sc0/a_ps/R65p = 2.
"""

import sys

if "/opt/trn_rl_repo" not in sys.path:
    sys.path.insert(0, "/opt/trn_rl_repo")

import numpy as np

import concourse.bacc as bacc
import concourse.bass as bass
import concourse.tile as tile
from concourse import bass_utils, mybir

F32 = mybir.dt.float32
F16 = mybir.dt.float16
BF16 = mybir.dt.bfloat16
I16 = mybir.dt.int16

C = 128          # channels
HW = 4096        # 64*64 pixels
L = 1024         # pooled keys (32*32)
D = 16           # attn dim
DV = 64          # value dim
KV = 80          # kv projection out width (v rows 0:64, k rows 64:80)
QB = 512         # q-block size
NB = HW // QB    # 8 q blocks
KC = 128         # keys per chunk
NCH = L // KC    # 8 key chunks
NCORES = 8
# wb: ident64 | wqt17 (rows 64:81) | bq (rows 64:80) | wfin | wkv
W16 = DV + C + 1 + C + KV
WF32 = 1                   # bkbq

# Schraudolph exp -> bf16 bits: bits = trunc(s * 2^7/ln2 + (127*2^7 - 4 + 0.5))
SCH_A = 128.0 / 0.6931471805599453
SCH_B = 16252.5


def build_kernel():
    nc = bacc.Bacc("TRN2", target_bir_lowering=False, debug=False)

    xh_d = nc.dram_tensor("xh", (C, HW), F16, kind="ExternalInput").ap()
    wb_d = nc.dram_tensor("wb", (C, W16), F16, kind="ExternalInput").ap()
    baux_d = nc.dram_tensor("baux", (C, WF32), F32, kind="ExternalInput").ap()
    out_d = nc.dram_tensor("out", (C, HW), F32, kind="ExternalOutput").ap()

    from contextlib import ExitStack

    with tile.TileContext(nc) as tc, ExitStack() as ctx:
        singles = ctx.enter_context(tc.tile_pool(name="singles", bufs=1))
        s1_pool = ctx.enter_context(tc.tile_pool(name="s1", bufs=4))
        attn_pool = ctx.enter_context(tc.tile_pool(name="attn", bufs=2))
        r_pool = ctx.enter_context(tc.tile_pool(name="r", bufs=2))
        ao_pool = ctx.enter_context(tc.tile_pool(name="ao", bufs=2))
        out_pool = ctx.enter_context(tc.tile_pool(name="outp", bufs=3))

        ps_sc = ctx.enter_context(tc.tile_pool(name="ps_sc", bufs=2, space="PSUM"))
        ps_av = ctx.enter_context(tc.tile_pool(name="ps_av", bufs=2, space="PSUM"))
        ps_fin = ctx.enter_context(tc.tile_pool(name="ps_fin", bufs=2, space="PSUM"))

        # ---- SBUF singles ----
        wb = singles.tile([C, W16], F16, tag="wb")
        xh = singles.tile([C, HW], F16, tag="xh")
        kvh = singles.tile([KV + 1, L], F16, tag="kvh")  # v 0:64, k 64:80, ones 80
        ab = singles.tile([C, L], F16, tag="ab")
        ones65 = singles.tile([1, DV + 1], BF16, tag="ones")
        baux = singles.tile([C, WF32], F32, tag="baux")

        identh = wb[0:DV, 0:DV]
        w_qt17 = wb[DV : KV + 1, DV : DV + C]            # rows 64:81
        b_q64 = wb[DV : DV + D, DV + C : DV + C + 1]     # rows 64:80
        w_fin = wb[0 : DV + 1, DV + C + 1 : DV + 2 * C + 1]
        w_kv = wb[:, DV + 2 * C + 1 : DV + 2 * C + 1 + KV]
        bkbq = baux[:, 0:1]

        # ---- input DMAs: dispatch from both DMA-capable hardware queues
        # (sync, scalar/ACT) in parallel; piece 0 + weights first ----
        nc.sync.dma_start(out=xh[:, 0:QB], in_=xh_d[:, 0:QB])
        nc.scalar.dma_start(out=wb, in_=wb_d)
        nc.sync.dma_start(out=xh[:, QB : 2 * QB], in_=xh_d[:, QB : 2 * QB])
        nc.scalar.dma_start(out=xh[:, 2 * QB : 3 * QB], in_=xh_d[:, 2 * QB : 3 * QB])
        nc.sync.dma_start(out=xh[:, 3 * QB : 4 * QB], in_=xh_d[:, 3 * QB : 4 * QB])
        nc.scalar.dma_start(out=xh[:, 4 * QB : 5 * QB], in_=xh_d[:, 4 * QB : 5 * QB])
        nc.sync.dma_start(out=xh[:, 5 * QB : 6 * QB], in_=xh_d[:, 5 * QB : 6 * QB])
        nc.scalar.dma_start(out=xh[:, 6 * QB : 7 * QB], in_=xh_d[:, 6 * QB : 7 * QB])
        nc.sync.dma_start(out=xh[:, 7 * QB : 8 * QB], in_=xh_d[:, 7 * QB : 8 * QB])
        nc.scalar.dma_start(out=baux, in_=baux_d)

        nc.vector.memset(ones65, 1.0)
        # ones row (partition 80) for the A-matmul bias fold; whole-tile
        # memset (start partition must be 0), pool overwrites rows 0:80
        nc.gpsimd.memset(kvh, 1.0)

        attn0 = attn_pool.tile([KC, NCH * QB], BF16, tag="attn")
        vt_t = ps_av.tile([C, QB], F32, tag="av")  # 8x(128,64) vT chunks
        vt16 = vt_t.bitcast(F16)
        bqk_t = ps_av.tile([C, QB], F32, tag="av")  # cols 0:8 used

        def late_tail(c):
            # A_c matmul (bias folded via kvh ones row), ab copy, block-0
            # scores + 512-wide exp. sc0 tiles live in ps_fin so the proj
            # ring (ps_sc) never waits on block-0 exps.
            csl = slice(c * KC, (c + 1) * KC)
            a_ps = ps_fin.tile([C, QB], F32, tag="fin", name=f"a{c}")
            nc.tensor.matmul(
                a_ps[:, 0:KC], lhsT=w_qt17, rhs=kvh[DV : KV + 1, csl],
                start=True, stop=True, tile_position=(DV, 0),
            )
            nc.vector.tensor_copy(ab[:, csl], a_ps[:, 0:KC])
            sc0c = ps_fin.tile([KC, QB], F32, tag="fin", name=f"sc0_{c}")
            nc.tensor.matmul(
                sc0c[:, :],
                lhsT=ab[:, csl],
                rhs=xh[:, 0:QB],
                start=True,
                stop=True,
            )
            nc.scalar.activation(
                attn0[:, c * QB : (c + 1) * QB],
                sc0c[:, :],
                mybir.ActivationFunctionType.Exp,
            )
            nc.tensor.matmul(
                bqk_t[:, c : c + 1], lhsT=kvh[DV : DV + D, csl], rhs=b_q64,
                start=True, stop=True, tile_position=(DV, 0),
            )
            nc.tensor.transpose(
                vt16[:, c * DV : (c + 1) * DV], kvh[0:DV, csl], identh
            )

        # ---- prologue: kv proj + pool chain, block-0 scores interleaved ----
        proj = None
        for c in range(NCH):
            j = c % 2
            if j == 0:
                proj = ps_sc.tile([KC, 2 * QB], F32, tag="sc", name=f"proj{c}")
            sl = slice(c * QB, (c + 1) * QB)
            nc.tensor.matmul(
                proj[:KV, j * QB : (j + 1) * QB],
                lhsT=w_kv,
                rhs=xh[:, sl],
                start=True,
                stop=True,
            )
            csl = slice(c * KC, (c + 1) * KC)
            # 2x2 maxpool via two DVE reduces (w-pairs from psum, then h-pairs)
            pv = proj[:KV, j * QB : (j + 1) * QB].rearrange(
                "p (w two) -> p w two", two=2
            )
            s1 = s1_pool.tile([KV, 256], F32, tag="s1")
            nc.vector.tensor_reduce(
                s1[:, :], pv, mybir.AxisListType.X, mybir.AluOpType.max
            )
            sv = s1.rearrange("p (h two w) -> p h w two", h=4, two=2)
            nc.vector.tensor_reduce(
                kvh[:KV, csl], sv, mybir.AxisListType.X, mybir.AluOpType.max
            )
            if c >= 1:
                late_tail(c - 1)
        late_tail(NCH - 1)
        # (defer_kv_aux is emitted right below, after its definition)

        ebqk = singles.tile([KC, NCH], F32, tag="ebqk")
        vaug = singles.tile([KC, NCH * (DV + 1)], BF16, tag="vaug")

        # ebqk + vaug assembly (vt/bqk matmuls already ran inside late_tail)
        nc.scalar.activation(
            ebqk[:, :], bqk_t[:, 0:NCH],
            mybir.ActivationFunctionType.Exp, bias=bkbq,
        )
        for c in range(NCH):
            base = c * (DV + 1)
            nc.vector.tensor_scalar_mul(
                vaug[:, base : base + DV],
                vt16[:, c * DV : (c + 1) * DV],
                ebqk[:, c : c + 1],
            )
            nc.vector.tensor_copy(
                vaug[:, base + DV : base + DV + 1], ebqk[:, c : c + 1]
            )

        # ---- main loop: 4-deep software pipeline (block 0 prefilled) ----
        # iter i: PE [sc(i) x8 | av(i-1) x8 | fin(i-3)]
        #         ACT [exp(i) tiles 0-2], DVE [schraudolph exp tile 3,
        #              dn+recip(i-1), ao-mul(i-2), residual-add(i-3)]
        #         DMA [r bounce (i-2), out (i-3)]
        attn_t, av_t, r_t, R65s_t, ao_t = {}, {}, {}, {}, {}
        attn_t[0] = attn0

        for i in range(1, NB + 3):
            b_sc = i          # scores + exp
            b_av = i - 1      # av accumulation + recip + bounce dispatch
            b_r = i - 2       # ao mul + fin + residual + store

            if b_sc < NB:
                qsl = slice(b_sc * QB, (b_sc + 1) * QB)
                attn = attn_pool.tile([KC, NCH * QB], BF16, tag="attn")
                attn_t[b_sc] = attn
                attn16 = attn.bitcast(I16)
                for t in range(4):
                    sc = ps_sc.tile([KC, 2 * QB], F32, tag="sc")
                    for j in range(2):
                        cc = 2 * t + j
                        nc.tensor.matmul(
                            sc[:, j * QB : (j + 1) * QB],
                            lhsT=ab[:, cc * KC : (cc + 1) * KC],
                            rhs=xh[:, qsl],
                            start=True,
                            stop=True,
                        )
                    # interleave av MMs of previous block between score tiles
                    if t == 1 and 0 <= b_av < NB:
                        _av_mms(nc, ps_av, av_t, vaug, attn_t, b_av, 0, 4)
                    if t == 2 and 0 <= b_av < NB:
                        _av_mms(nc, ps_av, av_t, vaug, attn_t, b_av, 4, 8)
                    if t > 0:
                        nc.scalar.activation(
                            attn[:, t * 2 * QB : (t + 1) * 2 * QB],
                            sc[:, :],
                            mybir.ActivationFunctionType.Exp,
                        )
                    else:
                        # Schraudolph exp on DVE: bf16 bits via int16 affine
                        nc.vector.tensor_scalar(
                            attn16[:, t * 2 * QB : (t + 1) * 2 * QB],
                            sc[:, :],
                            SCH_A,
                            SCH_B,
                            mybir.AluOpType.mult,
                            mybir.AluOpType.add,
                        )
                if b_sc == NB - 1:
                    # last block: start av(7) chunks 0-3 as soon as its first
                    # exps land (rest in the next iteration)
                    _av_mms(nc, ps_av, av_t, vaug, attn_t, b_sc, 0, 4)
            elif 0 <= b_av < NB:
                c0 = 4 if b_av == NB - 1 else 0
                _av_mms(nc, ps_av, av_t, vaug, attn_t, b_av, c0, 8)

            if 0 <= b_av < NB:
                # normalization: broadcast the bf16 denominator row over 65
                # partitions via PE ones-matmul, then one 65-partition recip
                # (free-size-driven cost). Replaces dn-copy+recip+rb+R65s.
                dnb = r_pool.tile([1, QB], BF16, tag="dn", name=f"dn{b_av}")
                nc.scalar.copy(dnb[:, :], av_t[b_av][DV : DV + 1, :])
                R65p = ps_fin.tile([C, QB], F32, tag="fin")
                nc.tensor.matmul(
                    R65p[0 : DV + 1, :], lhsT=ones65, rhs=dnb[:, :],
                    start=True, stop=True,
                )
                R65d = r_pool.tile([DV + 1, QB], F32, tag="R65d", name=f"Rd{b_av}")
                nc.scalar.copy(R65d[:, :], R65p[0 : DV + 1, :])
                R65s = r_pool.tile([DV + 1, QB], F32, tag="R65s", name=f"Rs{b_av}")
                nc.vector.reciprocal_approx_fast(R65s[:, :], R65d[:, :])
                R65s_t[b_av] = R65s

            if 0 <= b_r < NB:
                # ao + fin + residual + store in one stage (R65s ready since
                # the previous iteration); last two blocks in 256-col halves
                ao = ao_pool.tile([DV + 1, QB], F16, tag="ao")
                fin = ps_fin.tile([C, QB], F32, tag="fin")
                o = out_pool.tile([C, QB], F32, tag="o")
                if b_r >= NB - 2:
                    for h in range(2):
                        hs = slice(h * 256, (h + 1) * 256)
                        qh = slice(b_r * QB + h * 256, b_r * QB + (h + 1) * 256)
                        nc.vector.tensor_mul(
                            ao[:, hs], av_t[b_r][:, hs], R65s_t[b_r][:, hs]
                        )
                        nc.tensor.matmul(
                            fin[:, hs], lhsT=w_fin, rhs=ao[:, hs],
                            start=True, stop=True,
                        )
                        nc.vector.tensor_add(o[:, hs], fin[:, hs], xh[:, qh])
                        nc.sync.dma_start(out=out_d[:, qh], in_=o[:, hs])
                else:
                    qsl = slice(b_r * QB, (b_r + 1) * QB)
                    nc.vector.tensor_mul(
                        ao[:, :], av_t[b_r][:, :], R65s_t[b_r][:, :]
                    )
                    nc.tensor.matmul(
                        fin[:, :], lhsT=w_fin, rhs=ao[:, :],
                        start=True, stop=True,
                    )
                    nc.vector.tensor_add(o[:, :], fin[:, :], xh[:, qsl])
                    nc.sync.dma_start(out=out_d[:, qsl], in_=o[:, :])

    nc.compile()
    return nc


def _av_mms(nc, ps_av, av_t, vaug, attn_t, b, c0, c1):
    if b not in av_t:
        av_t[b] = ps_av.tile([DV + 1, QB], F32, tag="av", name=f"av{b}")
    av = av_t[b]
    attn = attn_t[b]
    for c in range(c0, c1):
        base = c * (DV + 1)
        nc.tensor.matmul(
            av[:, :],
            lhsT=vaug[:, base : base + DV + 1],
            rhs=attn[:, c * QB : (c + 1) * QB],
            start=(c == 0),
            stop=(c == NCH - 1),
        )


def prep_weights(Wq, bq, Wk, bk, Wv, bv, Wo, bo, gamma):
    g = np.float32(np.asarray(gamma))
    Wq, Wk, Wv, Wo = (np.asarray(a, np.float32) for a in (Wq, Wk, Wv, Wo))
    bq_, bk_, bv_, bo_ = (np.asarray(a, np.float32) for a in (bq, bk, bv, bo))
    wb = np.zeros((C, W16), np.float16)
    wb[0:DV, 0:DV] = np.eye(DV, dtype=np.float16)
    wb[DV : DV + D, DV : DV + C] = Wq.T.astype(np.float16)
    wb[KV, DV : DV + C] = (Wq @ bk_).astype(np.float16)  # bias row (ones fold)
    wb[DV : DV + D, DV + C] = bq_.astype(np.float16)
    wb[0:DV, DV + C + 1 : DV + 2 * C + 1] = (g * Wo).astype(np.float16)
    # bo' = bo + Wo.T bv  (v-bias folded host-side)
    wb[DV, DV + C + 1 : DV + 2 * C + 1] = (g * (bo_ + Wo.T @ bv_)).astype(
        np.float16
    )
    wb[:, DV + 2 * C + 1 : DV + 2 * C + 1 + DV] = Wv.astype(np.float16)
    wb[:, DV + 2 * C + 1 + DV : DV + 2 * C + 1 + KV] = Wk.astype(np.float16)
    baux = np.zeros((C, WF32), np.float32)
    baux[:, 0] = np.float32(bk_ @ bq_)  # bqk scalar bias
    return np.ascontiguousarray(wb), np.ascontiguousarray(baux)


_NC_CACHE = {}


def kernel(x, Wq, bq, Wk, bk, Wv, bv, Wo, bo, gamma):
    x = np.asarray(x, dtype=np.float32)
    N = x.shape[0]
    assert x.shape == (N, C, 64, 64) and N == NCORES
    wb, baux = prep_weights(Wq, bq, Wk, bk, Wv, bv, Wo, bo, gamma)

    if "nc" not in _NC_CACHE:
        _NC_CACHE["nc"] = build_kernel()
    nc = _NC_CACHE["nc"]

    in_maps = []
    for i in range(N):
        in_maps.append(
            {
                "xh": np.ascontiguousarray(x[i].reshape(C, HW).astype(np.float16)),
                "wb": wb,
                "baux": baux,
            }
        )
    res = bass_utils.run_bass_kernel_spmd(nc, in_maps, core_ids=list(range(N)))
    out = np.stack([res.results[i]["out"].reshape(C, 64, 64) for i in range(N)])
    return out.astype(np.float32)


if __name__ == "__main__":
    print("built", build_kernel())
